# revision 14
# baseline (speedup 1.0000x reference)
"""Causal self-attention (B=2, S=2048, E=1024, H=16, D=64) on 8 TRN2 NeuronCores.

Sharding: core c handles batch b = c//4 and head group hg = c%4 (4 heads).
Each core computes q/k/v projections for its heads, causal attention, and a
row-slice of the output projection; the host sums the 4 partial outputs per
batch and adds b_out.

Matmul operands are fp16 (full-rate PE + fast weight load); accumulation is
fp32 in PSUM. Layouts put every contraction on SBUF partitions:
  qkT   [128, 4, 2048]   Q cols (head pairs 01|23) then K cols, x tokens
  vaug  [128, 16, 4, 65] per tok-chunk, per head, [v | 1] (ones col -> softmax
                         sums appear as row 64 of the y-matmul output)
  yT    [128, 2, 512]    normalized head outputs packed for the out-projection
  outT  [1024, 2048]     partial (y @ W_out).T

Attention is pipelined per 128-token k-chunk: the two heads of a pair are
packed into one [128,1024] scores PSUM tile (even head -> PE rows 0-63, odd
head -> rows 64-127, running concurrently), exp'd in one ScalarE call, then
immediately consumed by the y-matmuls. Softmax normalization, out-projection
and the remaining QK projection tiles ride a deferred-work queue that drains
one item per chunk into the PE stream's ACT-wait gaps, keeping the PE dense
(and the HAM clock warm).
"""

import numpy as np

import concourse.bacc as bacc
import concourse.tile as tile
import concourse.mybir as mybir
from concourse.bass_utils import run_bass_kernel_spmd

B, S, E, H, D = 2, 2048, 1024, 16, 64
NH = 4           # heads per core
EC = NH * D      # 256 embedding cols per core
P = 128
TQ = 512         # q-tile (matmul free dim)
NT = S // TQ     # 4 q-tiles
NKC = S // P     # 16 k-chunks
NE = E // P      # 8 contraction chunks for projections
F16 = mybir.dt.float16
F32 = mybir.dt.float32
Exp = mybir.ActivationFunctionType.Exp
SCALE = float(1.0 / np.sqrt(D))

_prog_cache = {}


def _build():
    nc = bacc.Bacc("TRN2", target_bir_lowering=False, debug=False, num_devices=8)
    XT = nc.dram_tensor("xt", [E, S], F16, kind="ExternalInput")
    WQK = nc.dram_tensor("wqk", [E, 2 * EC], F16, kind="ExternalInput")
    WV = nc.dram_tensor("wv", [E, EC], F16, kind="ExternalInput")
    WO = nc.dram_tensor("wo", [EC, E], F16, kind="ExternalInput")
    BQK = nc.dram_tensor("bqk", [P, 4], F32, kind="ExternalInput")
    BV = nc.dram_tensor("bv", [P, 2], F32, kind="ExternalInput")
    TRI = nc.dram_tensor("tri", [P, P], F16, kind="ExternalInput")
    MSKB = nc.dram_tensor("mskb", [P, 4, TQ], F16, kind="ExternalInput")
    OUT = nc.dram_tensor("out", [E, S], F32, kind="ExternalOutput")

    with tile.TileContext(nc) as tc:
        with (
            tc.tile_pool(name="consts", bufs=1) as consts,
            tc.tile_pool(name="qkp", bufs=1) as qkp,
            tc.tile_pool(name="vp", bufs=1) as vp,
            tc.tile_pool(name="xp", bufs=1) as xp,
            tc.tile_pool(name="ytp", bufs=2) as ytp,
            tc.tile_pool(name="small", bufs=4) as small,
            tc.tile_pool(name="obp", bufs=3) as obp,
            tc.tile_pool(name="etp", bufs=4) as etp,
            tc.tile_pool(name="pgen", bufs=4, space="PSUM") as pgen,
            tc.tile_pool(name="pscore", bufs=2, space="PSUM") as pscore,
        ):
            # ---- input activations first: the xT DMA gates everything ----
            xT = xp.tile([P, NE, S], F16)
            XTr = XT[:].rearrange("(a p) t -> p a t", p=P)
            wv_sb = consts.tile([P, NE, EC], F16)
            nc.sync.dma_start(wv_sb[:], WV[:].rearrange("(a p) c -> p a c", p=P))
            for e in range(NE):
                dma_eng = nc.sync if e % 2 == 0 else nc.gpsimd
                dma_eng.dma_start(xT[:, e, :], XTr[:, e, :])

            # ---- constants ----
            wqk_sb = consts.tile([P, NE, 2 * EC], F16)
            nc.sync.dma_start(wqk_sb[:], WQK[:].rearrange("(a p) c -> p a c", p=P))
            wo_sb = consts.tile([P, EC // P, E], F16)
            nc.sync.dma_start(wo_sb[:], WO[:].rearrange("(a p) c -> p a c", p=P))
            bqk_sb = consts.tile([P, 4], F32)
            nc.sync.dma_start(bqk_sb[:], BQK[:])
            bv_sb = consts.tile([P, 2], F32)
            nc.sync.dma_start(bv_sb[:], BV[:])
            tri_sb = consts.tile([P, P], F16)
            nc.sync.dma_start(tri_sb[:], TRI[:])
            mskb_sb = consts.tile([P, 4, TQ], F16)
            nc.sync.dma_start(mskb_sb[:], MSKB[:])
            ones_f32 = consts.tile([P, 1], F32)
            nc.vector.memset(ones_f32[:], 1.0)
            ones_16 = consts.tile([1, D], F16)
            nc.vector.tensor_copy(ones_16[:], ones_f32[0:1, :].to_broadcast((1, D)))

            qkT = qkp.tile([P, 4, S], F16)
            vaug = vp.tile([P, NKC, NH, D + 1], F16)
            nc.vector.tensor_copy(
                vaug[:, :, :, D : D + 1], ones_f32[:].to_broadcast((P, NKC, NH, 1))
            )

            # ---- emit helpers ----
            def emit_qk_proj(tt, cc):
                pq = pgen.tile([P, TQ], F32, tag="mm")
                for e in range(NE):
                    nc.tensor.matmul(
                        pq[:],
                        wqk_sb[:, e, cc * P : (cc + 1) * P],
                        xT[:, e, tt * TQ : (tt + 1) * TQ],
                        start=(e == 0),
                        stop=(e == NE - 1),
                    )
                nc.vector.tensor_scalar_add(
                    qkT[:, cc, tt * TQ : (tt + 1) * TQ], pq[:], bqk_sb[:, cc : cc + 1]
                )

            def emit_flush(pyt, rc_r, yslot, bv_ap):
                pb = pgen.tile([P, TQ], F32, tag="mm")
                nc.tensor.matmul(
                    pb[0:D, :], ones_16[:], rc_r[:], start=True, stop=True
                )
                pbs = small.tile([D, TQ], F32)
                nc.vector.tensor_copy(pbs[:], pb[0:D, :])
                nc.vector.tensor_tensor(
                    yslot, pyt[0:D, :], pbs[:], mybir.AluOpType.mult
                )
                nc.vector.tensor_scalar_add(yslot, yslot, bv_ap)

            def emit_outproj(yT, t, eo):
                po = pgen.tile([P, TQ], F32, tag="mm")
                for a in range(EC // P):
                    nc.tensor.matmul(
                        po[:],
                        wo_sb[:, a, eo * P : (eo + 1) * P],
                        yT[:, a, :],
                        start=(a == 0),
                        stop=(a == EC // P - 1),
                    )
                ot = obp.tile([P, TQ], F32)
                nc.vector.tensor_copy(ot[:], po[:])
                nc.sync.dma_start(
                    OUT[eo * P : (eo + 1) * P, t * TQ : (t + 1) * TQ], ot[:]
                )

            from collections import deque

            flush_q = deque()  # normalizations: free pyt PSUM slots, run first
            projq = {tt: deque() for tt in range(NT)}  # V/QK tiles per q-tile
            opq = deque()  # out-projection tiles

            rr = [0]

            def pop_deferred(n=1, work_ok=True):
                for _ in range(n):
                    if flush_q:
                        flush_q.popleft()()
                        continue
                    if not work_ok:
                        continue
                    rr[0] ^= 1
                    srcs = [opq] if rr[0] else []
                    srcs += [projq[tt] for tt in range(NT)]
                    srcs += [] if rr[0] else [opq]
                    for q in srcs:
                        if q:
                            q.popleft()()
                            break

            def drain_flushes():
                while flush_q:
                    flush_q.popleft()()

            def emit_v_proj(c):
                pv = pgen.tile([P, TQ], F32, tag="mm")
                for e in range(NE):
                    nc.tensor.matmul(
                        pv[:, 0:EC],
                        xT[:, e, c * P : (c + 1) * P],
                        wv_sb[:, e, :],
                        start=(e == 0),
                        stop=(e == NE - 1),
                    )
                nc.scalar.copy(
                    vaug[:, c, :, 0:D],
                    pv[:, 0:EC].rearrange("p (h d) -> p h d", d=D),
                )

            # ---- upfront: QK(t0) interleaved with V chunks 0..3 ----
            for cc in range(4):
                emit_qk_proj(0, cc)
                emit_v_proj(cc)

            # ---- attention, chunk-pipelined, head-pair packed ----
            for t in range(NT):
                # correctness: pending flushes, old out-projections, and this
                # tile's projections must be emitted before its scores/y
                # matmuls enter the (in-order) PE queue
                drain_flushes()
                while opq:
                    opq.popleft()()
                while projq[t]:
                    projq[t].popleft()()
                if t + 1 < NT:
                    for cc in range(4):
                        projq[t + 1].append(lambda c=4 * (t + 1) + cc: emit_v_proj(c))
                        projq[t + 1].append(
                            lambda tt=t + 1, cc=cc: emit_qk_proj(tt, cc)
                        )
                yT = ytp.tile([P, EC // P, TQ], F16)
                for pair in range(2):
                    nchunks = 4 * (t + 1)
                    pyt_he = pgen.tile([P, TQ], F32, tag="mm")
                    pyt_ho = pgen.tile([P, TQ], F32, tag="mm")
                    ets = []
                    for c in range(nchunks):
                        psc = pscore.tile([P, 2 * TQ], F32, tag="sc")
                        jd = c - 4 * t
                        diag = jd >= 0
                        nm = P * (jd + 1) if diag else 0
                        nc.tensor.matmul(
                            psc[:, 0:TQ],
                            qkT[0:D, 2 + pair, c * P : (c + 1) * P],
                            qkT[0:D, pair, t * TQ : (t + 1) * TQ],
                            start=True,
                            stop=not diag,
                        )
                        if diag:  # accumulate -30000*count into invalid cols
                            nc.tensor.matmul(
                                psc[:, 0:nm],
                                tri_sb[:],
                                mskb_sb[:, jd, 0:nm],
                                start=False,
                                stop=True,
                            )
                        nc.tensor.matmul(
                            psc[:, TQ : 2 * TQ],
                            qkT[D:P, 2 + pair, c * P : (c + 1) * P],
                            qkT[D:P, pair, t * TQ : (t + 1) * TQ],
                            start=True,
                            stop=not diag,
                        )
                        if diag:
                            nc.tensor.matmul(
                                psc[:, TQ : TQ + nm],
                                tri_sb[:],
                                mskb_sb[:, jd, 0:nm],
                                start=False,
                                stop=True,
                            )
                        et = etp.tile([P, 2, TQ], F16, tag="et")
                        nc.scalar.activation(
                            et[:],
                            psc[:].rearrange("p (a q) -> p a q", a=2),
                            Exp,
                            scale=SCALE,
                        )
                        nc.tensor.matmul(
                            pyt_he[0 : D + 1, :],
                            vaug[:, c, 2 * pair, :],
                            et[:, 0, :],
                            start=(c == 0),
                            stop=(c == nchunks - 1),
                        )
                        nc.tensor.matmul(
                            pyt_ho[0 : D + 1, :],
                            vaug[:, c, 2 * pair + 1, :],
                            et[:, 1, :],
                            start=(c == 0),
                            stop=(c == nchunks - 1),
                        )
                        pop_deferred(work_ok=(c >= 2))
                    # reciprocals of the softmax sums (DVE, off the PE path)
                    for idx, pyt in ((0, pyt_he), (1, pyt_ho)):
                        h = 2 * pair + idx
                        pbase = D * idx
                        rc_s = small.tile([1, TQ], F32)
                        nc.vector.tensor_copy(rc_s[:], pyt[D : D + 1, :])
                        rc_f = small.tile([1, TQ], F32)
                        nc.vector.reciprocal_approx_fast(rc_f[:], rc_s[:])
                        rc_r = small.tile([1, TQ], F16)
                        nc.vector.tensor_copy(rc_r[:], rc_f[:])
                        yslot = yT[pbase : pbase + D, pair, :]
                        bv_ap = bv_sb[pbase : pbase + D, pair : pair + 1]
                        flush_q.append(
                            lambda pyt=pyt, rc_r=rc_r, yslot=yslot, bv_ap=bv_ap: (
                                emit_flush(pyt, rc_r, yslot, bv_ap)
                            )
                        )
                    pop_deferred()
                # out-projection for this q-tile rides the queue too
                for eo in range(NE):
                    opq.append(lambda yT=yT, t=t, eo=eo: emit_outproj(yT, t, eo))
            drain_flushes()
            while opq:
                opq.popleft()()

    nc.compile()
    return nc


def _shard_inputs(x, W_qkv, b_qkv, W_out, b_out):
    """Build the 8 per-core input maps (host-side layout preprocessing)."""
    Wq, Wk, Wv = W_qkv[:, 0:E], W_qkv[:, E : 2 * E], W_qkv[:, 2 * E : 3 * E]
    bq, bk, bv = b_qkv[0:E], b_qkv[E : 2 * E], b_qkv[2 * E : 3 * E]

    # causal mask as matmul operands: tri[r,p]=[r<=p]; mskb[r,j,f]=-30000*[r>f-128j]
    r = np.arange(P)
    tri = (r[:, None] <= r[None, :]).astype(np.float16)
    j = np.arange(4)[None, :, None]
    f = np.arange(TQ)[None, None, :]
    mskb = (-30000.0 * (r[:, None, None] > f - P * j)).astype(np.float16)

    in_maps = []
    for c in range(8):
        b, hg = c // 4, c % 4
        cs = slice(hg * EC, (hg + 1) * EC)
        in_maps.append(
            {
                "xt": np.ascontiguousarray(x[b].T.astype(np.float16)),
                "wqk": np.ascontiguousarray(
                    np.concatenate([Wq[:, cs], Wk[:, cs]], axis=1).astype(np.float16)
                ),
                "wv": np.ascontiguousarray(Wv[:, cs].astype(np.float16)),
                "wo": np.ascontiguousarray(W_out[cs, :].astype(np.float16)),
                "bqk": np.ascontiguousarray(
                    np.concatenate([bq[cs], bk[cs]]).reshape(4, P).T
                ),
                "bv": np.ascontiguousarray(bv[cs].reshape(2, P).T),
                "tri": tri,
                "mskb": mskb,
            }
        )
    return in_maps


def _run(inputs, trace=False):
    x = np.asarray(inputs["x"], dtype=np.float32)
    W_qkv = np.asarray(inputs["W_qkv"], dtype=np.float32)
    b_qkv = np.asarray(inputs["b_qkv"], dtype=np.float32)
    W_out = np.asarray(inputs["W_out"], dtype=np.float32)
    b_out = np.asarray(inputs["b_out"], dtype=np.float32)

    if "prog" not in _prog_cache:
        _prog_cache["prog"] = _build()
    nc = _prog_cache["prog"]

    in_maps = _shard_inputs(x, W_qkv, b_qkv, W_out, b_out)
    res = run_bass_kernel_spmd(nc, in_maps, core_ids=list(range(8)), trace=trace)

    out = np.zeros((B, S, E), dtype=np.float64)
    for c in range(8):
        out[c // 4] += res.results[c]["out"].astype(np.float64).T
    out += b_out.astype(np.float64)
    return out.astype(np.float32), res


def kernel(**inputs) -> np.ndarray:
    y, _ = _run(inputs, trace=False)
    return y


# revision 16
# speedup vs baseline: 1.0630x; 1.0630x over previous
"""Causal self-attention (B=2, S=2048, E=1024, H=16, D=64) on 8 TRN2 NeuronCores.

Sharding: core c handles batch b = c//4 and head group hg = c%4 (4 heads).
Each core computes q/k/v projections for its heads, causal attention, and a
row-slice of the output projection; the host sums the 4 partial outputs per
batch and adds b_out.

Matmul operands are fp16 (full-rate PE); accumulation is fp32 in PSUM.
Layouts put every matmul contraction on SBUF partitions:
  qkT   [128, 4, 2048]   Q cols (head pairs 01|23) then K cols, x tokens
  vaug  [128, 16, 4, 65] per tok-chunk, per head, [v | 1] (ones col -> softmax
                         sums appear as row 64 of the y-matmul output)
  yT    [128, 2, 512]    normalized head outputs packed for the out-projection
  outT  [1024, 2048]     partial (y @ W_out).T

Attention is pipelined per 128-token k-chunk: the two heads of a pair are
packed into one [128,1024] scores PSUM tile (even head -> PE rows 0-63, odd
head -> rows 64-127), exp'd in one ScalarE call, then immediately consumed by
the y-matmuls. The causal mask is applied INSIDE the scores accumulation
group as an extra matmul (tri.T @ big_negative) so masked logits underflow to
exactly 0 in the exp -- no vector-engine op on the critical path.

Work that is off the critical path -- softmax normalization (reciprocal +
PE-broadcast + scale), the out-projection, and the next q-tile's QK
projection -- rides a deferred queue drained one item per chunk into the PE
stream's exp-wait gaps. V-projection chunks are emitted just-in-time inside
the first pair that needs them (chunk c at q-tile t when c >= 4t).
"""

from collections import deque

import numpy as np

import concourse.bacc as bacc
import concourse.tile as tile
import concourse.mybir as mybir
from concourse.bass_utils import run_bass_kernel_spmd

B, S, E, H, D = 2, 2048, 1024, 16, 64
NH = 4           # heads per core
EC = NH * D      # 256 embedding cols per core
P = 128
TQ = 512         # q-tile (matmul free dim)
NT = S // TQ     # 4 q-tiles
NKC = S // P     # 16 k-chunks
NE = E // P      # 8 contraction chunks for projections
F16 = mybir.dt.float16
F32 = mybir.dt.float32
Exp = mybir.ActivationFunctionType.Exp
SCALE = float(1.0 / np.sqrt(D))

_prog_cache = {}


def _build():
    nc = bacc.Bacc("TRN2", target_bir_lowering=False, debug=False, num_devices=8)
    XT = nc.dram_tensor("xt", [E, S], F16, kind="ExternalInput")
    WQK = nc.dram_tensor("wqk", [E, 2 * EC], F16, kind="ExternalInput")
    WV = nc.dram_tensor("wv", [E, EC], F16, kind="ExternalInput")
    WO = nc.dram_tensor("wo", [EC, E], F16, kind="ExternalInput")
    BQK = nc.dram_tensor("bqk", [P, 4], F32, kind="ExternalInput")
    BV = nc.dram_tensor("bv", [P, 2], F32, kind="ExternalInput")
    TRI = nc.dram_tensor("tri", [P, P], F16, kind="ExternalInput")
    MSKB = nc.dram_tensor("mskb", [P, 4, TQ], F16, kind="ExternalInput")
    OUT = nc.dram_tensor("out", [E, S], F32, kind="ExternalOutput")

    with tile.TileContext(nc) as tc:
        with (
            tc.tile_pool(name="consts", bufs=1) as consts,
            tc.tile_pool(name="qkp", bufs=1) as qkp,
            tc.tile_pool(name="vp", bufs=1) as vp,
            tc.tile_pool(name="xp", bufs=1) as xp,
            tc.tile_pool(name="ytp", bufs=2) as ytp,
            tc.tile_pool(name="small", bufs=4) as small,
            tc.tile_pool(name="obp", bufs=3) as obp,
            tc.tile_pool(name="etp", bufs=4) as etp,
            tc.tile_pool(name="pgen", bufs=3, space="PSUM") as pgen,
            tc.tile_pool(name="pfl", bufs=1, space="PSUM") as pfl,
            tc.tile_pool(name="pscore", bufs=2, space="PSUM") as pscore,
        ):
            # ---- input DMAs: xT gates the projections, spread over 2 queues
            xT = xp.tile([P, NE, S], F16)
            XTr = XT[:].rearrange("(a p) t -> p a t", p=P)
            wqk_sb = consts.tile([P, NE, 2 * EC], F16)
            nc.sync.dma_start(wqk_sb[:], WQK[:].rearrange("(a p) c -> p a c", p=P))
            for e in range(NE):
                dma_eng = nc.sync if e % 2 == 0 else nc.gpsimd
                dma_eng.dma_start(xT[:, e, :], XTr[:, e, :])
            wv_sb = consts.tile([P, NE, EC], F16)
            nc.gpsimd.dma_start(wv_sb[:], WV[:].rearrange("(a p) c -> p a c", p=P))
            wo_sb = consts.tile([P, EC // P, E], F16)
            nc.sync.dma_start(wo_sb[:], WO[:].rearrange("(a p) c -> p a c", p=P))
            bqk_sb = consts.tile([P, 4], F32)
            nc.sync.dma_start(bqk_sb[:], BQK[:])
            bv_sb = consts.tile([P, 2], F32)
            nc.sync.dma_start(bv_sb[:], BV[:])
            tri_sb = consts.tile([P, P], F16)
            nc.sync.dma_start(tri_sb[:], TRI[:])
            mskb_sb = consts.tile([P, 4, TQ], F16)
            nc.sync.dma_start(mskb_sb[:], MSKB[:])
            ones_f32 = consts.tile([P, 1], F32)
            nc.vector.memset(ones_f32[:], 1.0)
            ones_16 = consts.tile([1, D], F16)
            nc.vector.tensor_copy(ones_16[:], ones_f32[0:1, :].to_broadcast((1, D)))

            qkT = qkp.tile([P, 4, S], F16)
            vaug = vp.tile([P, NKC, NH, D + 1], F16)
            nc.vector.tensor_copy(
                vaug[:, :, :, D : D + 1], ones_f32[:].to_broadcast((P, NKC, NH, 1))
            )

            # ---- emit helpers ----
            def emit_qk_proj(tt, cc):
                pq = pgen.tile([P, TQ], F32, tag="mm")
                for e in range(NE):
                    nc.tensor.matmul(
                        pq[:],
                        wqk_sb[:, e, cc * P : (cc + 1) * P],
                        xT[:, e, tt * TQ : (tt + 1) * TQ],
                        start=(e == 0),
                        stop=(e == NE - 1),
                    )
                nc.vector.tensor_scalar_add(
                    qkT[:, cc, tt * TQ : (tt + 1) * TQ], pq[:], bqk_sb[:, cc : cc + 1]
                )

            def emit_v_proj(c):
                pv = pgen.tile([P, TQ], F32, tag="mm")
                for e in range(NE):
                    nc.tensor.matmul(
                        pv[:, 0:EC],
                        xT[:, e, c * P : (c + 1) * P],
                        wv_sb[:, e, :],
                        start=(e == 0),
                        stop=(e == NE - 1),
                    )
                nc.scalar.copy(
                    vaug[:, c, :, 0:D],
                    pv[:, 0:EC].rearrange("p (h d) -> p h d", d=D),
                )

            def emit_flush(pyt, rc_r, yslot, bv_ap):
                pb = pfl.tile([P, TQ], F32, tag="pb")
                nc.tensor.matmul(
                    pb[0:D, :], ones_16[:], rc_r[:], start=True, stop=True
                )
                pbs = small.tile([D, TQ], F32)
                nc.vector.tensor_copy(pbs[:], pb[0:D, :])
                nc.vector.tensor_tensor(
                    yslot, pyt[0:D, :], pbs[:], mybir.AluOpType.mult
                )
                nc.vector.tensor_scalar_add(yslot, yslot, bv_ap)

            def emit_outproj(yT, t, eo):
                po = pgen.tile([P, TQ], F32, tag="mm")
                for a in range(EC // P):
                    nc.tensor.matmul(
                        po[:],
                        wo_sb[:, a, eo * P : (eo + 1) * P],
                        yT[:, a, :],
                        start=(a == 0),
                        stop=(a == EC // P - 1),
                    )
                ot = obp.tile([P, TQ], F32)
                nc.vector.tensor_copy(ot[:], po[:])
                nc.sync.dma_start(
                    OUT[eo * P : (eo + 1) * P, t * TQ : (t + 1) * TQ], ot[:]
                )

            flush_q = deque()  # normalizations: free pyt PSUM slots, run first
            work_q = deque()   # QK projection tiles / out-projection tiles

            def pop_deferred(n=1, work_ok=True):
                for _ in range(n):
                    if flush_q:
                        flush_q.popleft()()
                    elif work_q and work_ok:
                        work_q.popleft()()

            # ---- upfront: QK projection for q-tile 0 ----
            for cc in range(4):
                emit_qk_proj(0, cc)

            # ---- attention, chunk-pipelined, head-pair packed ----
            for t in range(NT):
                if t + 1 < NT:
                    for cc in range(4):
                        work_q.append(lambda tt=t + 1, cc=cc: emit_qk_proj(tt, cc))
                yT = ytp.tile([P, EC // P, TQ], F16)
                for pair in range(2):
                    nchunks = 4 * (t + 1)
                    pyt_he = pgen.tile([P, TQ], F32, tag="mm")
                    pyt_ho = pgen.tile([P, TQ], F32, tag="mm")
                    for c in range(nchunks):
                        psc = pscore.tile([P, 2 * TQ], F32, tag="sc")
                        jd = c - 4 * t
                        diag = jd >= 0
                        nm = P * (jd + 1) if diag else 0
                        nc.tensor.matmul(
                            psc[:, 0:TQ],
                            qkT[0:D, 2 + pair, c * P : (c + 1) * P],
                            qkT[0:D, pair, t * TQ : (t + 1) * TQ],
                            start=True,
                            stop=not diag,
                        )
                        if diag:  # causal mask: -30000*count on invalid cols
                            nc.tensor.matmul(
                                psc[:, 0:nm],
                                tri_sb[:],
                                mskb_sb[:, jd, 0:nm],
                                start=False,
                                stop=True,
                            )
                        nc.tensor.matmul(
                            psc[:, TQ : 2 * TQ],
                            qkT[D:P, 2 + pair, c * P : (c + 1) * P],
                            qkT[D:P, pair, t * TQ : (t + 1) * TQ],
                            start=True,
                            stop=not diag,
                        )
                        if diag:
                            nc.tensor.matmul(
                                psc[:, TQ : TQ + nm],
                                tri_sb[:],
                                mskb_sb[:, jd, 0:nm],
                                start=False,
                                stop=True,
                            )
                        et = etp.tile([P, 2, TQ], F16, tag="et")
                        nc.scalar.activation(
                            et[:],
                            psc[:].rearrange("p (a q) -> p a q", a=2),
                            Exp,
                            scale=SCALE,
                        )
                        if pair == 0 and c >= 4 * t:
                            # V chunks arrive just-in-time, filling exp waits
                            emit_v_proj(c)
                        nc.tensor.matmul(
                            pyt_he[0 : D + 1, :],
                            vaug[:, c, 2 * pair, :],
                            et[:, 0, :],
                            start=(c == 0),
                            stop=(c == nchunks - 1),
                        )
                        nc.tensor.matmul(
                            pyt_ho[0 : D + 1, :],
                            vaug[:, c, 2 * pair + 1, :],
                            et[:, 1, :],
                            start=(c == 0),
                            stop=(c == nchunks - 1),
                        )
                        pop_deferred(work_ok=(c >= 2))
                    # reciprocals of the softmax sums (DVE, off the PE path)
                    for idx, pyt in ((0, pyt_he), (1, pyt_ho)):
                        pbase = D * idx
                        rc_s = small.tile([1, TQ], F32)
                        nc.vector.tensor_copy(rc_s[:], pyt[D : D + 1, :])
                        rc_f = small.tile([1, TQ], F32)
                        nc.vector.reciprocal_approx_fast(rc_f[:], rc_s[:])
                        rc_r = small.tile([1, TQ], F16)
                        nc.vector.tensor_copy(rc_r[:], rc_f[:])
                        yslot = yT[pbase : pbase + D, pair, :]
                        bv_ap = bv_sb[pbase : pbase + D, pair : pair + 1]
                        flush_q.append(
                            lambda pyt=pyt, rc_r=rc_r, yslot=yslot, bv_ap=bv_ap: (
                                emit_flush(pyt, rc_r, yslot, bv_ap)
                            )
                        )
                # out-projection for this q-tile rides the queue too
                for eo in range(NE):
                    work_q.append(lambda yT=yT, t=t, eo=eo: emit_outproj(yT, t, eo))
            pop_deferred(len(flush_q) + len(work_q))

    nc.compile()
    return nc


def _shard_inputs(x, W_qkv, b_qkv, W_out, b_out):
    """Build the 8 per-core input maps (host-side layout preprocessing)."""
    Wq, Wk, Wv = W_qkv[:, 0:E], W_qkv[:, E : 2 * E], W_qkv[:, 2 * E : 3 * E]
    bq, bk, bv = b_qkv[0:E], b_qkv[E : 2 * E], b_qkv[2 * E : 3 * E]

    # causal mask as matmul operands: tri[r,p]=[r<=p]; mskb[r,j,f]=-30000*[r>f-128j]
    r = np.arange(P)
    tri = (r[:, None] <= r[None, :]).astype(np.float16)
    j = np.arange(4)[None, :, None]
    f = np.arange(TQ)[None, None, :]
    mskb = (-30000.0 * (r[:, None, None] > f - P * j)).astype(np.float16)

    in_maps = []
    for c in range(8):
        b, hg = c // 4, c % 4
        cs = slice(hg * EC, (hg + 1) * EC)
        in_maps.append(
            {
                "xt": np.ascontiguousarray(x[b].T.astype(np.float16)),
                "wqk": np.ascontiguousarray(
                    np.concatenate([Wq[:, cs], Wk[:, cs]], axis=1).astype(np.float16)
                ),
                "wv": np.ascontiguousarray(Wv[:, cs].astype(np.float16)),
                "wo": np.ascontiguousarray(W_out[cs, :].astype(np.float16)),
                "bqk": np.ascontiguousarray(
                    np.concatenate([bq[cs], bk[cs]]).reshape(4, P).T
                ),
                "bv": np.ascontiguousarray(bv[cs].reshape(2, P).T),
                "tri": tri,
                "mskb": mskb,
            }
        )
    return in_maps


def _run(inputs, trace=False):
    x = np.asarray(inputs["x"], dtype=np.float32)
    W_qkv = np.asarray(inputs["W_qkv"], dtype=np.float32)
    b_qkv = np.asarray(inputs["b_qkv"], dtype=np.float32)
    W_out = np.asarray(inputs["W_out"], dtype=np.float32)
    b_out = np.asarray(inputs["b_out"], dtype=np.float32)

    if "prog" not in _prog_cache:
        _prog_cache["prog"] = _build()
    nc = _prog_cache["prog"]

    in_maps = _shard_inputs(x, W_qkv, b_qkv, W_out, b_out)
    res = run_bass_kernel_spmd(nc, in_maps, core_ids=list(range(8)), trace=trace)

    out = np.zeros((B, S, E), dtype=np.float64)
    for c in range(8):
        out[c // 4] += res.results[c]["out"].astype(np.float64).T
    out += b_out.astype(np.float64)
    return out.astype(np.float32), res


def kernel(**inputs) -> np.ndarray:
    y, _ = _run(inputs, trace=False)
    return y


# revision 19
# speedup vs baseline: 1.1078x; 1.0422x over previous
"""Causal self-attention (B=2, S=2048, E=1024, H=16, D=64) on 8 TRN2 NeuronCores.

Sharding: core c handles batch b = c//4 and head group hg = c%4 (4 heads).
Each core computes q/k/v projections for its heads, causal attention, and a
row-slice of the output projection; the host sums the 4 partial outputs per
batch and adds b_out.

Matmul operands are fp16 (full-rate PE); accumulation is fp32 in PSUM.
Layouts put every matmul contraction on SBUF partitions:
  qkT   [128, 4, 2048]   Q cols (head pairs 01|23) then K cols, x tokens
  vaug  [128, 16, 4, 65] per tok-chunk, per head, [v | 1] (ones col -> softmax
                         sums appear as row 64 of the y-matmul output)
  yT    [128, 2, 512]    normalized head outputs packed for the out-projection
  outT  [1024, 2048]     partial (y @ W_out).T

Attention is pipelined per 128-token k-chunk: the two heads of a pair are
packed into one [128,1024] scores PSUM tile (even head -> PE rows 0-63, odd
head -> rows 64-127), exp'd in one ScalarE call, then immediately consumed by
the y-matmuls. The causal mask is applied INSIDE the scores accumulation
group as an extra matmul (tri.T @ big_negative) so masked logits underflow to
exactly 0 in the exp -- no vector-engine op on the critical path.

Work that is off the critical path -- softmax normalization (reciprocal +
PE-broadcast + scale), the out-projection, and the next q-tile's QK
projection -- rides a deferred queue drained one item per chunk into the PE
stream's exp-wait gaps. V-projection chunks are emitted just-in-time inside
the first pair that needs them (chunk c at q-tile t when c >= 4t).
"""

from collections import deque

import numpy as np

import concourse.bacc as bacc
import concourse.tile as tile
import concourse.mybir as mybir
from concourse.bass_utils import run_bass_kernel_spmd

B, S, E, H, D = 2, 2048, 1024, 16, 64
NH = 4           # heads per core
EC = NH * D      # 256 embedding cols per core
P = 128
TQ = 512         # q-tile (matmul free dim)
NT = S // TQ     # 4 q-tiles
NKC = S // P     # 16 k-chunks
NE = E // P      # 8 contraction chunks for projections
F16 = mybir.dt.float16
F32 = mybir.dt.float32
Exp = mybir.ActivationFunctionType.Exp
SCALE = float(1.0 / np.sqrt(D))

_prog_cache = {}


def _build():
    nc = bacc.Bacc("TRN2", target_bir_lowering=False, debug=False, num_devices=8)
    XT = nc.dram_tensor("xt", [E, S], F16, kind="ExternalInput")
    WQK = nc.dram_tensor("wqk", [E, 2 * EC], F16, kind="ExternalInput")
    WV = nc.dram_tensor("wv", [E, EC], F16, kind="ExternalInput")
    WO = nc.dram_tensor("wo", [EC, E], F16, kind="ExternalInput")
    BQK = nc.dram_tensor("bqk", [P, 4], F32, kind="ExternalInput")
    BV = nc.dram_tensor("bv", [P, 2], F32, kind="ExternalInput")
    MSK = nc.dram_tensor("msk", [P, 4, TQ], F16, kind="ExternalInput")
    OUT = nc.dram_tensor("out", [E, S], F32, kind="ExternalOutput")

    with tile.TileContext(nc) as tc:
        with (
            tc.tile_pool(name="consts", bufs=1) as consts,
            tc.tile_pool(name="qkp", bufs=1) as qkp,
            tc.tile_pool(name="vp", bufs=1) as vp,
            tc.tile_pool(name="xp", bufs=1) as xp,
            tc.tile_pool(name="ytp", bufs=2) as ytp,
            tc.tile_pool(name="small", bufs=4) as small,
            tc.tile_pool(name="obp", bufs=3) as obp,
            tc.tile_pool(name="etp", bufs=4) as etp,
            tc.tile_pool(name="pgen", bufs=4, space="PSUM") as pgen,
            tc.tile_pool(name="pscore", bufs=2, space="PSUM") as pscore,
        ):
            # ---- input DMAs: xT gates the projections, spread over 2 queues
            xT = xp.tile([P, NE, S], F16)
            XTr = XT[:].rearrange("(a p) t -> p a t", p=P)
            wqk_sb = consts.tile([P, NE, 2 * EC], F16)
            nc.sync.dma_start(wqk_sb[:], WQK[:].rearrange("(a p) c -> p a c", p=P))
            for e in range(NE):
                dma_eng = nc.sync if e % 2 == 0 else nc.gpsimd
                dma_eng.dma_start(xT[:, e, :], XTr[:, e, :])
            wv_sb = consts.tile([P, NE, EC], F16)
            nc.gpsimd.dma_start(wv_sb[:], WV[:].rearrange("(a p) c -> p a c", p=P))
            wo_sb = consts.tile([P, EC // P, E], F16)
            nc.sync.dma_start(wo_sb[:], WO[:].rearrange("(a p) c -> p a c", p=P))
            bqk_sb = consts.tile([P, 4], F32)
            nc.sync.dma_start(bqk_sb[:], BQK[:])
            bv_sb = consts.tile([P, 2], F32)
            nc.sync.dma_start(bv_sb[:], BV[:])
            msk_sb = consts.tile([P, 4, TQ], F16)
            nc.sync.dma_start(msk_sb[:], MSK[:])
            ones_f32 = consts.tile([P, 1], F32)
            nc.vector.memset(ones_f32[:], 1.0)
            ones_16 = consts.tile([1, D], F16)
            nc.vector.tensor_copy(ones_16[:], ones_f32[0:1, :].to_broadcast((1, D)))

            qkT = qkp.tile([P, 4, S], F16)
            vaug = vp.tile([P, NKC, NH, D + 1], F16)
            nc.vector.tensor_copy(
                vaug[:, :, :, D : D + 1], ones_f32[:].to_broadcast((P, NKC, NH, 1))
            )

            # ---- emit helpers ----
            def emit_qk_proj(tt, cc):
                pq = pgen.tile([P, TQ], F32, tag="mm")
                for e in range(NE):
                    nc.tensor.matmul(
                        pq[:],
                        wqk_sb[:, e, cc * P : (cc + 1) * P],
                        xT[:, e, tt * TQ : (tt + 1) * TQ],
                        start=(e == 0),
                        stop=(e == NE - 1),
                    )
                nc.vector.tensor_scalar_add(
                    qkT[:, cc, tt * TQ : (tt + 1) * TQ], pq[:], bqk_sb[:, cc : cc + 1]
                )

            def emit_v_proj(c):
                pv = pgen.tile([P, TQ], F32, tag="mm")
                for e in range(NE):
                    nc.tensor.matmul(
                        pv[:, 0:EC],
                        xT[:, e, c * P : (c + 1) * P],
                        wv_sb[:, e, :],
                        start=(e == 0),
                        stop=(e == NE - 1),
                    )
                nc.scalar.copy(
                    vaug[:, c, :, 0:D],
                    pv[:, 0:EC].rearrange("p (h d) -> p h d", d=D),
                )

            def emit_flush(pyt, rc_r, yslot, bv_ap):
                pb = pgen.tile([P, TQ], F32, tag="mm")
                nc.tensor.matmul(
                    pb[0:D, :], ones_16[:], rc_r[:], start=True, stop=True
                )
                pbs = small.tile([D, TQ], F32)
                nc.vector.tensor_copy(pbs[:], pb[0:D, :])
                nc.vector.tensor_tensor(
                    yslot, pyt[0:D, :], pbs[:], mybir.AluOpType.mult
                )
                nc.vector.tensor_scalar_add(yslot, yslot, bv_ap)

            def emit_outproj(yT, t, eo):
                po = pgen.tile([P, TQ], F32, tag="mm")
                for a in range(EC // P):
                    nc.tensor.matmul(
                        po[:],
                        wo_sb[:, a, eo * P : (eo + 1) * P],
                        yT[:, a, :],
                        start=(a == 0),
                        stop=(a == EC // P - 1),
                    )
                ot = obp.tile([P, TQ], F32)
                nc.vector.tensor_copy(ot[:], po[:])
                nc.sync.dma_start(
                    OUT[eo * P : (eo + 1) * P, t * TQ : (t + 1) * TQ], ot[:]
                )

            flush_q = deque()  # normalizations: free pyt PSUM slots, run first
            work_q = deque()   # QK projection tiles / out-projection tiles

            def pop_deferred(n=1, work_ok=True):
                for _ in range(n):
                    if flush_q:
                        flush_q.popleft()()
                    elif work_q and work_ok:
                        work_q.popleft()()

            # ---- upfront: V projection, then QK projection for q-tile 0 ----
            for c in range(NKC):
                emit_v_proj(c)
            for cc in range(4):
                emit_qk_proj(0, cc)

            # ---- attention, chunk-pipelined, head-pair packed ----
            for t in range(NT):
                if t + 1 < NT:
                    for cc in range(4):
                        work_q.append(lambda tt=t + 1, cc=cc: emit_qk_proj(tt, cc))
                yT = ytp.tile([P, EC // P, TQ], F16)
                for pair in range(2):
                    nchunks = 4 * (t + 1)
                    pyt_he = pgen.tile([P, TQ], F32, tag="mm")
                    pyt_ho = pgen.tile([P, TQ], F32, tag="mm")
                    for c in range(nchunks):
                        psc = pscore.tile([P, 2 * TQ], F32, tag="sc")
                        nc.tensor.matmul(
                            psc[:, 0:TQ],
                            qkT[0:D, 2 + pair, c * P : (c + 1) * P],
                            qkT[0:D, pair, t * TQ : (t + 1) * TQ],
                            start=True,
                            stop=True,
                        )
                        nc.tensor.matmul(
                            psc[:, TQ : 2 * TQ],
                            qkT[D:P, 2 + pair, c * P : (c + 1) * P],
                            qkT[D:P, pair, t * TQ : (t + 1) * TQ],
                            start=True,
                            stop=True,
                        )
                        et = etp.tile([P, 2, TQ], F16, tag="et")
                        nc.scalar.activation(
                            et[:],
                            psc[:].rearrange("p (a q) -> p a q", a=2),
                            Exp,
                            scale=SCALE,
                        )
                        jd = c - 4 * t
                        if jd >= 0:  # diagonal chunk: causal mask
                            nc.vector.tensor_tensor(
                                et[:],
                                et[:],
                                msk_sb[:, jd : jd + 1, :].to_broadcast((P, 2, TQ)),
                                mybir.AluOpType.mult,
                            )
                        nc.tensor.matmul(
                            pyt_he[0 : D + 1, :],
                            vaug[:, c, 2 * pair, :],
                            et[:, 0, :],
                            start=(c == 0),
                            stop=(c == nchunks - 1),
                        )
                        nc.tensor.matmul(
                            pyt_ho[0 : D + 1, :],
                            vaug[:, c, 2 * pair + 1, :],
                            et[:, 1, :],
                            start=(c == 0),
                            stop=(c == nchunks - 1),
                        )
                        pop_deferred(work_ok=(c >= 2))
                    # reciprocals of the softmax sums (DVE, off the PE path)
                    for idx, pyt in ((0, pyt_he), (1, pyt_ho)):
                        pbase = D * idx
                        rc_s = small.tile([1, TQ], F32)
                        nc.vector.tensor_copy(rc_s[:], pyt[D : D + 1, :])
                        rc_f = small.tile([1, TQ], F32)
                        nc.vector.reciprocal_approx_fast(rc_f[:], rc_s[:])
                        rc_r = small.tile([1, TQ], F16)
                        nc.vector.tensor_copy(rc_r[:], rc_f[:])
                        yslot = yT[pbase : pbase + D, pair, :]
                        bv_ap = bv_sb[pbase : pbase + D, pair : pair + 1]
                        flush_q.append(
                            lambda pyt=pyt, rc_r=rc_r, yslot=yslot, bv_ap=bv_ap: (
                                emit_flush(pyt, rc_r, yslot, bv_ap)
                            )
                        )
                    pop_deferred()
                # out-projection for this q-tile rides the queue too
                for eo in range(NE):
                    work_q.append(lambda yT=yT, t=t, eo=eo: emit_outproj(yT, t, eo))
            pop_deferred(len(flush_q) + len(work_q))

    nc.compile()
    return nc


def _shard_inputs(x, W_qkv, b_qkv, W_out, b_out):
    """Build the 8 per-core input maps (host-side layout preprocessing)."""
    Wq, Wk, Wv = W_qkv[:, 0:E], W_qkv[:, E : 2 * E], W_qkv[:, 2 * E : 3 * E]
    bq, bk, bv = b_qkv[0:E], b_qkv[E : 2 * E], b_qkv[2 * E : 3 * E]

    # causal mask for the 4 diagonal 128-chunks of a 512-wide q-tile
    p = np.arange(P)[:, None, None]
    j = np.arange(4)[None, :, None]
    f = np.arange(TQ)[None, None, :]
    msk = (p + P * j <= f).astype(np.float16)

    in_maps = []
    for c in range(8):
        b, hg = c // 4, c % 4
        cs = slice(hg * EC, (hg + 1) * EC)
        in_maps.append(
            {
                "xt": np.ascontiguousarray(x[b].T.astype(np.float16)),
                "wqk": np.ascontiguousarray(
                    np.concatenate([Wq[:, cs], Wk[:, cs]], axis=1).astype(np.float16)
                ),
                "wv": np.ascontiguousarray(Wv[:, cs].astype(np.float16)),
                "wo": np.ascontiguousarray(W_out[cs, :].astype(np.float16)),
                "bqk": np.ascontiguousarray(
                    np.concatenate([bq[cs], bk[cs]]).reshape(4, P).T
                ),
                "bv": np.ascontiguousarray(bv[cs].reshape(2, P).T),
                "msk": msk,
            }
        )
    return in_maps


def _run(inputs, trace=False):
    x = np.asarray(inputs["x"], dtype=np.float32)
    W_qkv = np.asarray(inputs["W_qkv"], dtype=np.float32)
    b_qkv = np.asarray(inputs["b_qkv"], dtype=np.float32)
    W_out = np.asarray(inputs["W_out"], dtype=np.float32)
    b_out = np.asarray(inputs["b_out"], dtype=np.float32)

    if "prog" not in _prog_cache:
        _prog_cache["prog"] = _build()
    nc = _prog_cache["prog"]

    in_maps = _shard_inputs(x, W_qkv, b_qkv, W_out, b_out)
    res = run_bass_kernel_spmd(nc, in_maps, core_ids=list(range(8)), trace=trace)

    out = np.zeros((B, S, E), dtype=np.float64)
    for c in range(8):
        out[c // 4] += res.results[c]["out"].astype(np.float64).T
    out += b_out.astype(np.float64)
    return out.astype(np.float32), res


def kernel(**inputs) -> np.ndarray:
    y, _ = _run(inputs, trace=False)
    return y


# revision 20
# speedup vs baseline: 1.1389x; 1.0280x over previous
"""Causal self-attention (B=2, S=2048, E=1024, H=16, D=64) on 8 TRN2 NeuronCores.

Sharding: core c handles batch b = c//4 and head group hg = c%4 (4 heads).
Each core computes q/k/v projections for its heads, causal attention, and a
row-slice of the output projection; the host sums the 4 partial outputs per
batch and adds b_out.

Matmul operands are fp16 (full-rate PE); accumulation is fp32 in PSUM.
Layouts put every matmul contraction on SBUF partitions:
  qkT   [128, 4, 2048]   Q cols (head pairs 01|23) then K cols, x tokens
  vaug  [128, 16, 4, 65] per tok-chunk, per head, [v | 1] (ones col -> softmax
                         sums appear as row 64 of the y-matmul output)
  yT    [128, 2, 512]    normalized head outputs packed for the out-projection
  outT  [1024, 2048]     partial (y @ W_out).T

Attention is pipelined per 128-token k-chunk: the two heads of a pair are
packed into one [128,1024] scores PSUM tile (even head -> PE rows 0-63, odd
head -> rows 64-127), exp'd in one ScalarE call, then immediately consumed by
the y-matmuls. The causal mask is applied INSIDE the scores accumulation
group as an extra matmul (tri.T @ big_negative) so masked logits underflow to
exactly 0 in the exp -- no vector-engine op on the critical path.

Work that is off the critical path -- softmax normalization (reciprocal +
PE-broadcast + scale), the out-projection, and the next q-tile's QK
projection -- rides a deferred queue drained one item per chunk into the PE
stream's exp-wait gaps. V-projection chunks are emitted just-in-time inside
the first pair that needs them (chunk c at q-tile t when c >= 4t).
"""

from collections import deque

import numpy as np

import concourse.bacc as bacc
import concourse.tile as tile
import concourse.mybir as mybir
from concourse.bass_utils import run_bass_kernel_spmd

B, S, E, H, D = 2, 2048, 1024, 16, 64
NH = 4           # heads per core
EC = NH * D      # 256 embedding cols per core
P = 128
TQ = 512         # q-tile (matmul free dim)
NT = S // TQ     # 4 q-tiles
NKC = S // P     # 16 k-chunks
NE = E // P      # 8 contraction chunks for projections
F16 = mybir.dt.float16
F32 = mybir.dt.float32
Exp = mybir.ActivationFunctionType.Exp
SCALE = float(1.0 / np.sqrt(D))

_prog_cache = {}


def _build():
    nc = bacc.Bacc("TRN2", target_bir_lowering=False, debug=False, num_devices=8)
    XT = nc.dram_tensor("xt", [E, S], F16, kind="ExternalInput")
    WQK = nc.dram_tensor("wqk", [E, 2 * EC], F16, kind="ExternalInput")
    WV = nc.dram_tensor("wv", [E, EC], F16, kind="ExternalInput")
    WO = nc.dram_tensor("wo", [EC, E], F16, kind="ExternalInput")
    BQK = nc.dram_tensor("bqk", [P, 4], F32, kind="ExternalInput")
    BV = nc.dram_tensor("bv", [P, 2], F32, kind="ExternalInput")
    MSK = nc.dram_tensor("msk", [P, 4, TQ], F16, kind="ExternalInput")
    OUT = nc.dram_tensor("out", [E, S], F32, kind="ExternalOutput")

    with tile.TileContext(nc) as tc:
        with (
            tc.tile_pool(name="consts", bufs=1) as consts,
            tc.tile_pool(name="qkp", bufs=1) as qkp,
            tc.tile_pool(name="vp", bufs=1) as vp,
            tc.tile_pool(name="xp", bufs=1) as xp,
            tc.tile_pool(name="ytp", bufs=2) as ytp,
            tc.tile_pool(name="small", bufs=4) as small,
            tc.tile_pool(name="obp", bufs=3) as obp,
            tc.tile_pool(name="etp", bufs=4) as etp,
            tc.tile_pool(name="pgen", bufs=4, space="PSUM") as pgen,
            tc.tile_pool(name="pscore", bufs=2, space="PSUM") as pscore,
        ):
            # ---- input DMAs: xT gates the projections, spread over 2 queues
            xT = xp.tile([P, NE, S], F16)
            XTr = XT[:].rearrange("(a p) t -> p a t", p=P)
            wv_sb = consts.tile([P, NE, EC], F16)
            nc.sync.dma_start(wv_sb[:], WV[:].rearrange("(a p) c -> p a c", p=P))
            for e in range(NE):
                dma_eng = nc.sync if e % 2 == 0 else nc.gpsimd
                dma_eng.dma_start(xT[:, e, :], XTr[:, e, :])
            wqk_sb = consts.tile([P, NE, 2 * EC], F16)
            nc.sync.dma_start(wqk_sb[:], WQK[:].rearrange("(a p) c -> p a c", p=P))
            wo_sb = consts.tile([P, EC // P, E], F16)
            nc.sync.dma_start(wo_sb[:], WO[:].rearrange("(a p) c -> p a c", p=P))
            bqk_sb = consts.tile([P, 4], F32)
            nc.sync.dma_start(bqk_sb[:], BQK[:])
            bv_sb = consts.tile([P, 2], F32)
            nc.sync.dma_start(bv_sb[:], BV[:])
            msk_sb = consts.tile([P, 4, TQ], F16)
            nc.sync.dma_start(msk_sb[:], MSK[:])
            ones_f32 = consts.tile([P, 1], F32)
            nc.vector.memset(ones_f32[:], 1.0)
            ones_16 = consts.tile([1, D], F16)
            nc.vector.tensor_copy(ones_16[:], ones_f32[0:1, :].to_broadcast((1, D)))

            qkT = qkp.tile([P, 4, S], F16)
            vaug = vp.tile([P, NKC, NH, D + 1], F16)
            nc.vector.tensor_copy(
                vaug[:, :, :, D : D + 1], ones_f32[:].to_broadcast((P, NKC, NH, 1))
            )

            # ---- emit helpers ----
            def emit_qk_proj(tt, cc):
                pq = pgen.tile([P, TQ], F32, tag="mm")
                for e in range(NE):
                    nc.tensor.matmul(
                        pq[:],
                        wqk_sb[:, e, cc * P : (cc + 1) * P],
                        xT[:, e, tt * TQ : (tt + 1) * TQ],
                        start=(e == 0),
                        stop=(e == NE - 1),
                    )
                nc.vector.tensor_scalar_add(
                    qkT[:, cc, tt * TQ : (tt + 1) * TQ], pq[:], bqk_sb[:, cc : cc + 1]
                )

            def emit_v_proj(c):
                pv = pgen.tile([P, TQ], F32, tag="mm")
                for e in range(NE):
                    nc.tensor.matmul(
                        pv[:, 0:EC],
                        xT[:, e, c * P : (c + 1) * P],
                        wv_sb[:, e, :],
                        start=(e == 0),
                        stop=(e == NE - 1),
                    )
                nc.scalar.copy(
                    vaug[:, c, :, 0:D],
                    pv[:, 0:EC].rearrange("p (h d) -> p h d", d=D),
                )

            def emit_flush(pyt, rc_r, yslot, bv_ap):
                pb = pgen.tile([P, TQ], F32, tag="mm")
                nc.tensor.matmul(
                    pb[0:D, :], ones_16[:], rc_r[:], start=True, stop=True
                )
                pbs = small.tile([D, TQ], F32)
                nc.vector.tensor_copy(pbs[:], pb[0:D, :])
                nc.vector.tensor_tensor(
                    yslot, pyt[0:D, :], pbs[:], mybir.AluOpType.mult
                )
                nc.vector.tensor_scalar_add(yslot, yslot, bv_ap)

            def emit_outproj(yT, t, eo):
                po = pgen.tile([P, TQ], F32, tag="mm")
                for a in range(EC // P):
                    nc.tensor.matmul(
                        po[:],
                        wo_sb[:, a, eo * P : (eo + 1) * P],
                        yT[:, a, :],
                        start=(a == 0),
                        stop=(a == EC // P - 1),
                    )
                ot = obp.tile([P, TQ], F32)
                nc.vector.tensor_copy(ot[:], po[:])
                nc.sync.dma_start(
                    OUT[eo * P : (eo + 1) * P, t * TQ : (t + 1) * TQ], ot[:]
                )

            flush_q = deque()  # normalizations: free pyt PSUM slots, run first
            work_q = deque()   # QK projection tiles / out-projection tiles

            def pop_deferred(n=1, work_ok=True):
                for _ in range(n):
                    if flush_q:
                        flush_q.popleft()()
                    elif work_q and work_ok:
                        work_q.popleft()()

            # ---- upfront: V projection, then QK projection for q-tile 0 ----
            for c in range(NKC):
                emit_v_proj(c)
            for cc in range(4):
                emit_qk_proj(0, cc)

            # ---- attention, chunk-pipelined, head-pair packed ----
            for t in range(NT):
                if t + 1 < NT:
                    for cc in range(4):
                        work_q.append(lambda tt=t + 1, cc=cc: emit_qk_proj(tt, cc))
                yT = ytp.tile([P, EC // P, TQ], F16)
                for pair in range(2):
                    nchunks = 4 * (t + 1)
                    pyt_he = pgen.tile([P, TQ], F32, tag="mm")
                    pyt_ho = pgen.tile([P, TQ], F32, tag="mm")
                    for c in range(nchunks):
                        psc = pscore.tile([P, 2 * TQ], F32, tag="sc")
                        nc.tensor.matmul(
                            psc[:, 0:TQ],
                            qkT[0:D, 2 + pair, c * P : (c + 1) * P],
                            qkT[0:D, pair, t * TQ : (t + 1) * TQ],
                            start=True,
                            stop=True,
                        )
                        nc.tensor.matmul(
                            psc[:, TQ : 2 * TQ],
                            qkT[D:P, 2 + pair, c * P : (c + 1) * P],
                            qkT[D:P, pair, t * TQ : (t + 1) * TQ],
                            start=True,
                            stop=True,
                        )
                        et = etp.tile([P, 2, TQ], F16, tag="et")
                        nc.scalar.activation(
                            et[:],
                            psc[:].rearrange("p (a q) -> p a q", a=2),
                            Exp,
                            scale=SCALE,
                        )
                        jd = c - 4 * t
                        if jd >= 0:  # diagonal chunk: causal mask
                            nc.vector.tensor_tensor(
                                et[:],
                                et[:],
                                msk_sb[:, jd : jd + 1, :].to_broadcast((P, 2, TQ)),
                                mybir.AluOpType.mult,
                            )
                        nc.tensor.matmul(
                            pyt_he[0 : D + 1, :],
                            vaug[:, c, 2 * pair, :],
                            et[:, 0, :],
                            start=(c == 0),
                            stop=(c == nchunks - 1),
                        )
                        nc.tensor.matmul(
                            pyt_ho[0 : D + 1, :],
                            vaug[:, c, 2 * pair + 1, :],
                            et[:, 1, :],
                            start=(c == 0),
                            stop=(c == nchunks - 1),
                        )
                        pop_deferred(work_ok=(c >= 2))
                    # reciprocals of the softmax sums (DVE, off the PE path)
                    for idx, pyt in ((0, pyt_he), (1, pyt_ho)):
                        pbase = D * idx
                        rc_s = small.tile([1, TQ], F32)
                        nc.vector.tensor_copy(rc_s[:], pyt[D : D + 1, :])
                        rc_f = small.tile([1, TQ], F32)
                        nc.vector.reciprocal_approx_fast(rc_f[:], rc_s[:])
                        rc_r = small.tile([1, TQ], F16)
                        nc.vector.tensor_copy(rc_r[:], rc_f[:])
                        yslot = yT[pbase : pbase + D, pair, :]
                        bv_ap = bv_sb[pbase : pbase + D, pair : pair + 1]
                        flush_q.append(
                            lambda pyt=pyt, rc_r=rc_r, yslot=yslot, bv_ap=bv_ap: (
                                emit_flush(pyt, rc_r, yslot, bv_ap)
                            )
                        )
                    pop_deferred()
                # out-projection for this q-tile rides the queue too
                for eo in range(NE):
                    work_q.append(lambda yT=yT, t=t, eo=eo: emit_outproj(yT, t, eo))
            pop_deferred(len(flush_q) + len(work_q))

    nc.compile()
    return nc


def _shard_inputs(x, W_qkv, b_qkv, W_out, b_out):
    """Build the 8 per-core input maps (host-side layout preprocessing)."""
    Wq, Wk, Wv = W_qkv[:, 0:E], W_qkv[:, E : 2 * E], W_qkv[:, 2 * E : 3 * E]
    bq, bk, bv = b_qkv[0:E], b_qkv[E : 2 * E], b_qkv[2 * E : 3 * E]

    # causal mask for the 4 diagonal 128-chunks of a 512-wide q-tile
    p = np.arange(P)[:, None, None]
    j = np.arange(4)[None, :, None]
    f = np.arange(TQ)[None, None, :]
    msk = (p + P * j <= f).astype(np.float16)

    in_maps = []
    for c in range(8):
        b, hg = c // 4, c % 4
        cs = slice(hg * EC, (hg + 1) * EC)
        in_maps.append(
            {
                "xt": np.ascontiguousarray(x[b].T.astype(np.float16)),
                "wqk": np.ascontiguousarray(
                    np.concatenate([Wq[:, cs], Wk[:, cs]], axis=1).astype(np.float16)
                ),
                "wv": np.ascontiguousarray(Wv[:, cs].astype(np.float16)),
                "wo": np.ascontiguousarray(W_out[cs, :].astype(np.float16)),
                "bqk": np.ascontiguousarray(
                    np.concatenate([bq[cs], bk[cs]]).reshape(4, P).T
                ),
                "bv": np.ascontiguousarray(bv[cs].reshape(2, P).T),
                "msk": msk,
            }
        )
    return in_maps


def _run(inputs, trace=False):
    x = np.asarray(inputs["x"], dtype=np.float32)
    W_qkv = np.asarray(inputs["W_qkv"], dtype=np.float32)
    b_qkv = np.asarray(inputs["b_qkv"], dtype=np.float32)
    W_out = np.asarray(inputs["W_out"], dtype=np.float32)
    b_out = np.asarray(inputs["b_out"], dtype=np.float32)

    if "prog" not in _prog_cache:
        _prog_cache["prog"] = _build()
    nc = _prog_cache["prog"]

    in_maps = _shard_inputs(x, W_qkv, b_qkv, W_out, b_out)
    res = run_bass_kernel_spmd(nc, in_maps, core_ids=list(range(8)), trace=trace)

    out = np.zeros((B, S, E), dtype=np.float64)
    for c in range(8):
        out[c // 4] += res.results[c]["out"].astype(np.float64).T
    out += b_out.astype(np.float64)
    return out.astype(np.float32), res


def kernel(**inputs) -> np.ndarray:
    y, _ = _run(inputs, trace=False)
    return y
